# revision 31
# baseline (speedup 1.0000x reference)
"""Trainium2 Bass kernel for a 2-layer GCN + 2-layer MLP (gnn_message_passing).

Model (see reference):
    h1 = relu(GCNConv(x;  W1, b1))       # symmetric-normalized, self-loops
    h2 = relu(GCNConv(h1; W2, b2))
    h3 = relu(h2 @ Wl1 + bl1)
    y  = h3 @ Wl2 + bl2                  # [N, 1]

Distribution: nodes are RELABELED by a host-chosen permutation into 8 shards
of NPC slots; each core aggregates the edges whose destination it owns.  Per
layer each core computes the scaled table T = dinv * (h @ W) for its shard,
the shards are AllGathered into t_c [NPAD, 32] fp16, and messages T[src] are
fetched per edge with SWDGE dma_gather.

Key layout tricks vs. a naive port:
  * The compact table IS 4 interleaved 256B-strided subtables: rows q::4 have
    a 256-byte stride, so gathers read t_c directly (no strided "expand"
    copy); an edge's subtable is slot(src) % 4 and its index slot(src) // 4,
    which fits int16 because NPAD/4 < 32768.
  * Edge slot grid: buckets keyed by (dst-block of 64, src%4) with a uniform
    capacity of BCAP slots (BCOLS columns of 128).  The host rebalances the
    node->slot permutation (swapping equal-residue nodes between blocks) so
    no bucket overflows.  Self-loops are NOT in the grid: the self term
    dinv[d]*(T[d]) is added from a sequential read of the core's own shard.
  * One PSUM tile [64, GPB, 32] accumulates a whole group (GPB blocks); the
    per-block scatter one-hot [128, 64] is built on DVE from dstloc vs iota.
  * Layer-1 tail work (scale/bias/relu/transpose + the T2 table build) and
    the final MLP run per-group, interleaved with the gathers.
"""

import math
import sys

import numpy as np

sys.path.insert(0, "/opt/trn_rl_repo")
sys.path.insert(0, "/root/problem")

import concourse.bass as bass
import concourse.mybir as mybir
import concourse.ap_utils as ap_utils
import concourse.tile as tile
from concourse import bacc
from concourse._compat import exact_div
from concourse.bass_utils import run_bass_kernel_spmd


def dma_gather_raw(eng, out_ap, in_ap, idxs_ap, num_idxs, num_idxs_reg,
                   elem_size, elem_step, single_packet=False, queue_num=0):
    """gathered = in[idxs, :elem_size]; rows strided elem_step elements.

    Clone of BassGpSimd.dma_gather's HBM path minus the
    `elem_size_bytes % 256 == 0` restriction (the Q7 ucode only requires the
    row STRIDE to be a 256-byte multiple; payload bytes are free)."""
    assert idxs_ap.dtype == mybir.dt.int16
    assert in_ap.dtype == out_ap.dtype
    dt_size = mybir.dt.size(in_ap.dtype)
    assert ap_utils.ap_is_contiguous(out_ap.ap[1:])
    assert ap_utils.ap_is_contiguous(idxs_ap.ap[1:])
    assert in_ap.ap[-1][1] == out_ap.ap[-1][1] == elem_size
    assert out_ap.ap[0][1] * out_ap.ap[1][1] == ((num_idxs + 127) // 128) * 128
    assert in_ap.ap[0][0] == elem_step
    stride_bytes_256 = exact_div(elem_step * dt_size, 256)
    assert stride_bytes_256 < 256

    _in_ap = eng.lower_ap_dma(in_ap, for_custom_bir_dma=True)
    _idxs_ap = eng.lower_ap(idxs_ap)
    _out_ap = eng.lower_ap(out_ap)
    return eng.add_instruction(
        mybir.InstDMAGatherAnt(
            name=eng.bass.get_next_instruction_name(),
            ins=[*_in_ap, _idxs_ap,
                 eng.lower_val_access(eng.to_reg(num_idxs_reg))],
            outs=[_out_ap],
            transpose=False,
            num_idxs=num_idxs,
            elem_size=elem_size,
            stride_bytes_256=stride_bytes_256,
            gen_mode=0,
            single_packet=single_packet,
            queue_num=queue_num,
            sbuf_tokens_per_rank=0,
            sbuf_free_dim_per_rank=0,
            sbuf_free_dim_pad_per_rank=0,
            sbuf_byte_offset=0,
        )
    )


FP16 = mybir.dt.float16
FP32 = mybir.dt.float32
INT16 = mybir.dt.int16
Alu = mybir.AluOpType
Act = mybir.ActivationFunctionType

N_CORES = 8
MD = 64            # dst-block size
NQ = 4             # src subtables (slot % 4); NPAD/4 must fit int16
GPB = 7            # dst-blocks per group


class Cfg:
    def __init__(self, n_nodes, in_ch, hid, bcols):
        self.in_ch = in_ch
        self.hid = hid
        self.bcols = bcols
        self.n_cores = N_CORES
        npc = math.ceil(n_nodes / N_CORES)
        lcm = math.lcm(128, MD)
        npc = ((npc + lcm - 1) // lcm) * lcm
        self.npc = npc
        self.npad = npc * N_CORES
        assert self.npad % NQ == 0
        self.qsz = self.npad // NQ
        assert self.qsz <= 32768
        self.nb = npc // MD                    # dst blocks per core
        self.ngroups = math.ceil(self.nb / GPB)
        self.bcap = bcols * 128                # slots per bucket
        self.ncols = self.nb * NQ * bcols      # grid columns per core
        self.ntile128 = npc // 128
        self.ntile128f = self.npad // 128

    def group_blocks(self, g):
        b0 = g * GPB
        return b0, min(GPB, self.nb - b0)


def host_prep(x, edge_index, W1, b1, W2, b2, Wl1, bl1, Wl2, bl2):
    n = x.shape[0]
    in_ch = x.shape[1]
    hid = W1.shape[1]
    cfg = Cfg(n, in_ch, hid, 8)
    npc, npad, nb = cfg.npc, cfg.npad, cfg.nb

    loops = np.arange(n, dtype=np.int64)
    src = np.concatenate([np.asarray(edge_index[0], dtype=np.int64), loops])
    dst = np.concatenate([np.asarray(edge_index[1], dtype=np.int64), loops])
    srcq = (src % NQ).astype(np.int64)

    # node -> slot permutation. Start with identity (residue-preserving) and
    # swap same-residue nodes between blocks until no (block, quarter) bucket
    # exceeds the cap (8 columns; fall back to 9 if balancing stalls).
    slot_of = np.arange(npad, dtype=np.int64)       # node -> slot (pads tail)
    # per-node in-edge quarter profile
    prof = np.zeros((npad, NQ), dtype=np.int64)
    np.add.at(prof, (dst, srcq), 1)

    blk = slot_of[dst] // MD
    cnt = np.bincount(blk * NQ + srcq,
                      minlength=npad // MD * NQ).reshape(-1, NQ)
    node_of = np.arange(npad, dtype=np.int64)       # slot -> node
    cap = cfg.bcap
    for _round in range(40000):
        over_b, over_q = np.nonzero(cnt > cap)
        if len(over_b) == 0:
            break
        bo, q = over_b[0], over_q[0]
        # donor: node in block bo with largest q-profile
        nodes_bo = node_of[np.arange(bo * MD, (bo + 1) * MD)]
        dn = nodes_bo[np.argmax(prof[nodes_bo, q])]
        r = dn % NQ
        # recipient: among 32 lightest-in-q blocks, min worst-quarter load
        nb_all = cnt.shape[0]
        cand_b = np.argpartition(cnt[:, q], min(32, nb_all - 1))[:32]
        worst = (cnt[cand_b] + prof[dn]).max(axis=1)
        br = int(cand_b[np.argmin(worst)])
        if br == bo:
            cap = None
            break
        nodes_br = node_of[np.arange(br * MD, (br + 1) * MD)]
        cand = np.nonzero(nodes_br % NQ == r)[0]
        cn = nodes_br[cand[np.argmin(prof[nodes_br[cand], q])]]
        s1, s2 = slot_of[dn], slot_of[cn]
        slot_of[dn], slot_of[cn] = s2, s1
        node_of[s1], node_of[s2] = cn, dn
        cnt[bo] += prof[cn] - prof[dn]
        cnt[br] += prof[dn] - prof[cn]
    bcols = int(np.ceil(cnt.max() / 128))
    if bcols != cfg.bcols:
        cfg = Cfg(n, in_ch, hid, bcols)
    BCOLS = cfg.bcols
    assert (cnt <= cfg.bcap).all()

    sdst = slot_of[dst]
    ssrc = slot_of[src]
    gq = ssrc % NQ
    gidx_val = (ssrc // NQ).astype(np.int16)
    core = sdst // npc
    blk_l = (sdst % npc) // MD
    dloc_val = (sdst % MD).astype(np.float16)

    # order edges by (core, block, quarter); fill bucket slots sequentially
    key = (core * nb + blk_l) * NQ + gq
    order = np.argsort(key, kind="stable")
    key_s = key[order]
    gidx_s = gidx_val[order]
    dloc_s = dloc_val[order]
    cnts = np.bincount(key_s, minlength=N_CORES * nb * NQ)
    ofs = np.concatenate([[0], np.cumsum(cnts)])

    # column index of bucket (b, q) within its core's grid:
    #   group g = b // GPB; within group: q-major then block then col
    def bucket_col0(b, q):
        g = b // GPB
        b0, nbg = cfg.group_blocks(g)
        base = b0 * NQ * BCOLS
        return base + q * (nbg * BCOLS) + (b - b0) * BCOLS

    gidx_all, dstloc_all = [], []
    for c in range(N_CORES):
        gsl = np.zeros((128, cfg.ncols), dtype=np.int16)
        dloc = np.full((128, cfg.ncols), 10000.0, dtype=np.float16)
        for b in range(nb):
            for q in range(NQ):
                k = (c * nb + b) * NQ + q
                s, e = ofs[k], ofs[k + 1]
                if e == s:
                    continue
                col0 = bucket_col0(b, q)
                sl = np.arange(e - s)
                p_i, t_i = sl % 128, sl // 128
                gsl[p_i, col0 + t_i] = gidx_s[s:e]
                dloc[p_i, col0 + t_i] = dloc_s[s:e]
        # int16 idx stream: 16-partition wrapped, replicated x8
        flat = gsl.T.reshape(-1)
        w = flat.reshape(-1, 16)
        idxw = np.zeros((128, cfg.ncols * 8), dtype=np.int16)
        for g8 in range(8):
            idxw[g8 * 16:(g8 + 1) * 16, :] = w.T
        gidx_all.append(idxw)
        dstloc_all.append(dloc)

    # degree per slot (in-degree incl. self loop); pads get deg 1
    deg = np.zeros(npad, dtype=np.float32)
    np.add.at(deg, sdst, 1.0)
    deg[deg == 0] = 1.0
    dinv = 1.0 / np.sqrt(deg)

    xf = np.zeros((npad, in_ch), dtype=np.float32)
    xf[slot_of[:n]] = np.asarray(x, dtype=np.float32)

    dinvT_all, dinv128_all = [], []
    for c in range(N_CORES):
        d = dinv[c * npc:(c + 1) * npc]
        dinvT_all.append(np.tile(d.astype(np.float16)[None, :], (hid, 1)))
        dinv128_all.append(np.ascontiguousarray(
            d.reshape(cfg.ntile128, 128).T).astype(np.float32))
    xTf = np.ascontiguousarray(xf.T).astype(np.float16)
    dinv128f = np.ascontiguousarray(
        dinv.reshape(cfg.ntile128f, 128).T).astype(np.float32)

    iota = np.tile(np.arange(MD, dtype=np.float16)[None, :], (128, 1))

    consts = {
        "W1": np.asarray(W1, np.float16),
        "W2": np.asarray(W2, np.float16),
        "Wl1": np.asarray(Wl1, np.float16),
        "Wl2": np.asarray(Wl2, np.float16),
        "b1c": np.asarray(b1, np.float32).reshape(hid, 1),
        "b2c": np.asarray(b2, np.float32).reshape(hid, 1),
        "bl1": np.asarray(bl1, np.float32).reshape(hid, 1),
        "bl2": np.asarray(bl2, np.float32).reshape(1, 1),
        "iotaM": iota,
        "xT": xTf,
        "dinv128f": dinv128f,
    }
    in_maps = []
    for c in range(N_CORES):
        m = dict(consts)
        m["gidx"] = gidx_all[c]
        m["dstloc"] = dstloc_all[c]
        m["dinvT"] = dinvT_all[c]
        m["dinv128"] = dinv128_all[c]
        in_maps.append(m)
    return cfg, in_maps, node_of


def build_program(cfg: Cfg):
    nc = bacc.Bacc("TRN2", target_bir_lowering=False, num_swdge_queues=4)
    H, NPC, NB = cfg.hid, cfg.npc, cfg.nb
    NPAD, BCOLS = cfg.npad, cfg.bcols
    GCMAX = GPB * NQ * BCOLS

    xT_d = nc.dram_tensor("xT", [cfg.in_ch, NPAD], FP16, kind="ExternalInput")
    W1_d = nc.dram_tensor("W1", [cfg.in_ch, H], FP16, kind="ExternalInput")
    W2_d = nc.dram_tensor("W2", [H, H], FP16, kind="ExternalInput")
    Wl1_d = nc.dram_tensor("Wl1", [H, H], FP16, kind="ExternalInput")
    Wl2_d = nc.dram_tensor("Wl2", [H, 1], FP16, kind="ExternalInput")
    b1c_d = nc.dram_tensor("b1c", [H, 1], FP32, kind="ExternalInput")
    b2c_d = nc.dram_tensor("b2c", [H, 1], FP32, kind="ExternalInput")
    bl1_d = nc.dram_tensor("bl1", [H, 1], FP32, kind="ExternalInput")
    bl2_d = nc.dram_tensor("bl2", [1, 1], FP32, kind="ExternalInput")
    dinvT_d = nc.dram_tensor("dinvT", [H, NPC], FP16, kind="ExternalInput")
    dinv128_d = nc.dram_tensor("dinv128", [128, cfg.ntile128], FP32,
                               kind="ExternalInput")
    dinv128f_d = nc.dram_tensor("dinv128f", [128, cfg.ntile128f], FP32,
                                kind="ExternalInput")
    gidx_d = nc.dram_tensor("gidx", [128, cfg.ncols * 8], INT16,
                            kind="ExternalInput")
    dstloc_d = nc.dram_tensor("dstloc", [128, cfg.ncols], FP16,
                              kind="ExternalInput")
    iota_d = nc.dram_tensor("iotaM", [128, MD], FP16, kind="ExternalInput")
    y_d = nc.dram_tensor("y", [NPC], FP32, kind="ExternalOutput")

    t2s_d = nc.dram_tensor("t2s", [NPC, H], FP16)
    t1c_d = nc.dram_tensor("t1c", [NPAD, H], FP16)
    t2c_d = nc.dram_tensor("t2c", [NPAD, H], FP16, addr_space="Shared")

    dstloc_s = nc.alloc_sbuf_tensor("dstloc_s", [128, cfg.ncols], FP16).ap()
    iota_s = nc.alloc_sbuf_tensor("iota_s", [128, MD], FP16).ap()
    W2_s = nc.alloc_sbuf_tensor("W2_s", [H, H], FP16).ap()
    Wl1_s = nc.alloc_sbuf_tensor("Wl1_s", [H, H], FP16).ap()
    Wl2_s = nc.alloc_sbuf_tensor("Wl2_s", [H, 1], FP16).ap()
    b1c_s = nc.alloc_sbuf_tensor("b1c_s", [H, 1], FP32).ap()
    b2c_s = nc.alloc_sbuf_tensor("b2c_s", [H, 1], FP32).ap()
    bl1_s = nc.alloc_sbuf_tensor("bl1_s", [H, 1], FP32).ap()
    bl2_s = nc.alloc_sbuf_tensor("bl2_s", [1, 1], FP32).ap()
    dinv128_s = nc.alloc_sbuf_tensor("dinv128_s", [128, cfg.ntile128],
                                     FP32).ap()
    dinv128f_s = nc.alloc_sbuf_tensor("dinv128f_s", [128, cfg.ntile128f],
                                      FP32).ap()

    cc_sem = nc.alloc_semaphore("cc_sem")

    # subtable views: rows q::4 of t_c -> stride NQ*H elems (256B), offset q*H
    def subtable(tc_d, q):
        return tc_d[:].rearrange("(i q) e -> i (q e)", q=NQ)[
            :, q * H:(q + 1) * H]

    def make_table_tiles(pool, psum, hT, W_s, ts_d, j0, j1):
        """T[j0*128:(j1)*128] = fp16(dinv * (h @ W)) -> ts_d rows."""
        for ja in range(j0, j1, 4):
            jb = min(ja + 4, j1)
            k = jb - ja
            ps = psum.tile([128, 4, H], FP32, tag="tbl_ps")
            for j in range(ja, jb):
                nc.tensor.matmul(ps[:, j - ja, :],
                                 hT[:, j * 128:(j + 1) * 128], W_s[:],
                                 start=True, stop=True)
            ts = pool.tile([128, 4, H], FP16, tag="tbl_sb")
            dv = dinv128_s[:, ja:jb].rearrange("p (j a) -> p j a", a=1)
            nc.vector.tensor_tensor(ts[:, :k, :], ps[:, :k, :],
                                    dv.to_broadcast([128, k, H]), Alu.mult)
            nc.sync.dma_start(
                ts_d[ja * 128:jb * 128, :].rearrange(
                    "(j p) e -> p j e", p=128), ts[:, :k, :])

    def agg_layer(gpool, bigpool, pool, psum, tc_d, bc_s, hT_out, layer):
        """Aggregate one GCN layer group-by-group; fused tails."""
        wm = 0  # T2 watermark in 128-node tiles (layer 1 only)
        for g in range(cfg.ngroups):
            b0, nbg = cfg.group_blocks(g)
            gc = nbg * NQ * BCOLS            # grid cols this group
            col0 = b0 * NQ * BCOLS
            qcols = nbg * BCOLS              # cols per quarter
            nodes0 = b0 * MD
            gn = nbg * MD                    # nodes this group

            idxb = gpool.tile([128, GPB * NQ * BCOLS * 8], INT16, tag="idx")
            nc.sync.dma_start(idxb[:, :gc * 8],
                              gidx_d[:, col0 * 8:(col0 + gc) * 8])
            msgs = []
            for q in range(NQ):
                mt = gpool.tile([128, GPB * BCOLS, H], FP16, tag=f"msg{q}")
                msgs.append(mt)
                dma_gather_raw(
                    nc.gpsimd, mt[:, :qcols, :], subtable(tc_d, q),
                    idxb[:, q * qcols * 8:(q + 1) * qcols * 8],
                    qcols * 128, qcols * 128, H, NQ * H,
                    queue_num=(g * NQ + q) % 4)

            # scatter one-hot, column-major: oh[p, c, d] = (dstloc[p,c] == d)
            oh = bigpool.tile([128, GPB * NQ * BCOLS, MD], FP16, tag="oh")
            dl = dstloc_s[:, col0:col0 + gc]
            dl3 = dl.rearrange("p (t a) -> p t a", a=1).to_broadcast(
                [128, gc, MD])
            io3 = iota_s[:].rearrange("p (a d) -> p a d", a=1).to_broadcast(
                [128, gc, MD])
            nc.vector.tensor_tensor(oh[:, :gc, :], dl3, io3, Alu.is_equal)

            # out[f, d] += msgs[s, f]^T @ oh[s, d], accumulated per block
            ps = psum.tile([H, GPB, MD], FP32, tag="agg_ps")
            for bl in range(nbg):
                im = 0
                nmm = NQ * BCOLS
                for q in range(NQ):
                    for t in range(BCOLS):
                        ct = q * qcols + bl * BCOLS + t
                        nc.tensor.matmul(ps[:, bl, :],
                                         msgs[q][:, bl * BCOLS + t, :],
                                         oh[:, ct, :],
                                         start=(im == 0), stop=(im == nmm - 1))
                        im += 1

            # tail: hT = relu(dinv_d * sum + b)  (self loop is in the grid)
            dvt = pool.tile([H, GPB * MD], FP16, tag="dvt")
            nc.sync.dma_start(dvt[:, :gn], dinvT_d[:, nodes0:nodes0 + gn])
            tsum = pool.tile([H, GPB * MD], FP32, tag="tsum")
            nc.vector.tensor_tensor(
                tsum[:, :gn],
                ps[:].rearrange("f b d -> f (b d)")[:, :gn],
                dvt[:, :gn], Alu.mult)
            nc.scalar.activation(hT_out[:, nodes0:nodes0 + gn],
                                 tsum[:, :gn], Act.Relu, bias=bc_s[:, 0:1])

            if layer == 1:
                # T2 for completed 128-node tiles
                hi = (nodes0 + gn) // 128
                if hi > wm:
                    make_table_tiles(pool, psum, hT_out, W2_s, t2s_d, wm, hi)
                    wm = hi
            else:
                # MLP tail for this group's nodes
                zp = psum.tile([H, GPB * MD], FP32, tag="zps")
                nc.tensor.matmul(zp[:, :gn], Wl1_s[:],
                                 hT_out[:, nodes0:nodes0 + gn],
                                 start=True, stop=True)
                zt = pool.tile([H, GPB * MD], FP16, tag="zt")
                nc.scalar.activation(zt[:, :gn], zp[:, :gn], Act.Relu,
                                     bias=bl1_s[:, 0:1])
                yp = psum.tile([1, GPB * MD], FP32, tag="yps")
                nc.tensor.matmul(yp[:, :gn], Wl2_s[:], zt[:, :gn],
                                 start=True, stop=True)
                ys = pool.tile([1, GPB * MD], FP32, tag="ysl")
                nc.scalar.activation(ys[:, :gn], yp[:, :gn], Act.Identity,
                                     bias=bl2_s[:, 0:1])
                y2 = y_d[:].rearrange("(a n) -> a n", a=1)
                nc.sync.dma_start(y2[:, nodes0:nodes0 + gn], ys[:, :gn])

    # ------- Phase 1: constants; every core builds the FULL T1 table -------
    with tile.TileContext(nc) as tc:
        with tc.tile_pool(name="p1", bufs=2) as pool, \
             tc.tile_pool(name="p1ps", bufs=2, space="PSUM") as psum:
            nc.sync.dma_start(dstloc_s[:], dstloc_d[:])
            nc.sync.dma_start(iota_s[:], iota_d[:])
            nc.sync.dma_start(W2_s[:], W2_d[:])
            nc.sync.dma_start(Wl1_s[:], Wl1_d[:])
            nc.sync.dma_start(Wl2_s[:], Wl2_d[:])
            nc.sync.dma_start(b1c_s[:], b1c_d[:])
            nc.sync.dma_start(b2c_s[:], b2c_d[:])
            nc.sync.dma_start(bl1_s[:], bl1_d[:])
            nc.sync.dma_start(bl2_s[:], bl2_d[:])

            nc.sync.dma_start(dinv128_s[:], dinv128_d[:])
            nc.sync.dma_start(dinv128f_s[:], dinv128f_d[:])

            W1 = pool.tile([cfg.in_ch, H], FP16)
            nc.sync.dma_start(W1[:], W1_d[:])
            CH = 2048
            for x0 in range(0, NPAD, CH):
                xT = pool.tile([cfg.in_ch, CH], FP16, tag="xT")
                nc.sync.dma_start(xT[:], xT_d[:, x0:x0 + CH])
                for ja in range(0, CH // 128, 4):
                    jg = x0 // 128 + ja
                    ps = psum.tile([128, 4, H], FP32, tag="t1_ps")
                    for j in range(4):
                        nc.tensor.matmul(
                            ps[:, j, :],
                            xT[:, (ja + j) * 128:(ja + j + 1) * 128],
                            W1[:], start=True, stop=True)
                    ts = pool.tile([128, 4, H], FP16, tag="t1_sb")
                    dv = dinv128f_s[:, jg:jg + 4].rearrange(
                        "p (j a) -> p j a", a=1)
                    nc.vector.tensor_tensor(ts[:], ps[:],
                                            dv.to_broadcast([128, 4, H]),
                                            Alu.mult)
                    nc.sync.dma_start(
                        t1c_d[jg * 128:(jg + 4) * 128, :].rearrange(
                            "(j p) e -> p j e", p=128), ts[:])

    # ---------------- Phase 2: aggregate L1 (+T2 build) ----------------
    with tile.TileContext(nc) as tc:
        with tc.tile_pool(name="p2b", bufs=2) as bigpool, \
             tc.tile_pool(name="p2g", bufs=3) as gpool, \
             tc.tile_pool(name="p2", bufs=3) as pool, \
             tc.tile_pool(name="p2h", bufs=1) as hpool, \
             tc.tile_pool(name="p2ps", bufs=2, space="PSUM") as psum:
            h1T = hpool.tile([H, NPC], FP16)
            agg_layer(gpool, bigpool, pool, psum, t1c_d, b1c_s, h1T, layer=1)

    nc.gpsimd.collective_compute(
        "AllGather", Alu.bypass, replica_groups=[list(range(N_CORES))],
        ins=[t2s_d[:]], outs=[t2c_d[:]]).then_inc(cc_sem, 1)
    nc.gpsimd.wait_ge(cc_sem, 1)

    # ---------------- Phase 3: aggregate L2 + MLP ----------------
    with tile.TileContext(nc) as tc:
        with tc.tile_pool(name="p3b", bufs=2) as bigpool, \
             tc.tile_pool(name="p3g", bufs=3) as gpool, \
             tc.tile_pool(name="p3", bufs=3) as pool, \
             tc.tile_pool(name="p3h", bufs=1) as hpool, \
             tc.tile_pool(name="p3ps", bufs=2, space="PSUM") as psum:
            h2T = hpool.tile([H, NPC], FP16)
            agg_layer(gpool, bigpool, pool, psum, t2c_d, b2c_s, h2T, layer=2)

    nc.compile()
    return nc


_CACHE = {}


def _get_program(key, cfg):
    if key not in _CACHE:
        _CACHE[key] = build_program(cfg)
    return _CACHE[key]


def kernel(x, edge_index, W1, b1, W2, b2, Wl1, bl1, Wl2, bl2):
    x = np.asarray(x)
    n = x.shape[0]
    cfg, in_maps, node_of = host_prep(x, edge_index, W1, b1, W2, b2,
                                      Wl1, bl1, Wl2, bl2)
    key = (n, cfg.in_ch, cfg.hid, cfg.bcols)
    nc = _get_program(key, cfg)
    res = run_bass_kernel_spmd(nc, in_maps, list(range(N_CORES)))
    ys = [res.results[c]["y"].reshape(-1) for c in range(N_CORES)]
    y_slots = np.concatenate(ys)
    # slot s holds output of node node_of[s]
    y = np.empty(n, dtype=np.float32)
    valid = node_of < n
    y[node_of[valid]] = y_slots[valid]
    return y.reshape(n, 1)


# revision 33
# speedup vs baseline: 1.5734x; 1.5734x over previous
"""Trainium2 Bass kernel for a 2-layer GCN + 2-layer MLP (gnn_message_passing).

Model (see reference):
    h1 = relu(GCNConv(x;  W1, b1))       # symmetric-normalized, self-loops
    h2 = relu(GCNConv(h1; W2, b2))
    h3 = relu(h2 @ Wl1 + bl1)
    y  = h3 @ Wl2 + bl2                  # [N, 1]

Distribution: nodes are RELABELED by a host-chosen permutation into 8 shards
of NPC slots; each core aggregates the edges whose destination it owns.  Per
layer each core computes the scaled table T = dinv * (h @ W) for its shard,
the shards are AllGathered into t_c [NPAD, 32] fp16, and messages T[src] are
fetched per edge with SWDGE dma_gather.

Key layout tricks vs. a naive port:
  * The compact table IS 4 interleaved 256B-strided subtables: rows q::4 have
    a 256-byte stride, so gathers read t_c directly (no strided "expand"
    copy); an edge's subtable is slot(src) % 4 and its index slot(src) // 4,
    which fits int16 because NPAD/4 < 32768.
  * Edge slot grid: buckets keyed by (dst-block of 64, src%4) with a uniform
    capacity of BCAP slots (BCOLS columns of 128).  The host rebalances the
    node->slot permutation (swapping equal-residue nodes between blocks) so
    no bucket overflows.  Self-loop edges live in the grid like any edge.
  * The scatter matmul runs "flipped": lhsT = the gathered messages column
    (contiguous [128, 32] weights), rhs = the one-hot column (contiguous
    [128, 64]), so the output lands feat-major [32, 64] in PSUM and no
    transposes are needed; one PSUM tile [32, GPB, 64] holds a whole group.
  * Tail = one DVE multiply by a replicated dinv row + one ScalarE
    bias+relu activation straight into hT.  The T2 table build (layer 1)
    and the final MLP (layer 2) run per-group, interleaved with gathers.
  * Per-descriptor cost of SWDGE gather is ~2.4 ns regardless of payload
    (HW-measured); total slots is the wall, so padding is minimized.
"""

import math
import sys

import numpy as np

sys.path.insert(0, "/opt/trn_rl_repo")
sys.path.insert(0, "/root/problem")

import concourse.bass as bass
import concourse.mybir as mybir
import concourse.ap_utils as ap_utils
import concourse.tile as tile
from concourse import bacc
from concourse._compat import exact_div
from concourse.bass_utils import run_bass_kernel_spmd


def dma_gather_raw(eng, out_ap, in_ap, idxs_ap, num_idxs, num_idxs_reg,
                   elem_size, elem_step, single_packet=False, queue_num=0):
    """gathered = in[idxs, :elem_size]; rows strided elem_step elements.

    Clone of BassGpSimd.dma_gather's HBM path minus the
    `elem_size_bytes % 256 == 0` restriction (the Q7 ucode only requires the
    row STRIDE to be a 256-byte multiple; payload bytes are free)."""
    assert idxs_ap.dtype == mybir.dt.int16
    assert in_ap.dtype == out_ap.dtype
    dt_size = mybir.dt.size(in_ap.dtype)
    assert ap_utils.ap_is_contiguous(out_ap.ap[1:])
    assert ap_utils.ap_is_contiguous(idxs_ap.ap[1:])
    assert in_ap.ap[-1][1] == out_ap.ap[-1][1] == elem_size
    assert out_ap.ap[0][1] * out_ap.ap[1][1] == ((num_idxs + 127) // 128) * 128
    assert in_ap.ap[0][0] == elem_step
    stride_bytes_256 = exact_div(elem_step * dt_size, 256)
    assert stride_bytes_256 < 256

    _in_ap = eng.lower_ap_dma(in_ap, for_custom_bir_dma=True)
    _idxs_ap = eng.lower_ap(idxs_ap)
    _out_ap = eng.lower_ap(out_ap)
    return eng.add_instruction(
        mybir.InstDMAGatherAnt(
            name=eng.bass.get_next_instruction_name(),
            ins=[*_in_ap, _idxs_ap,
                 eng.lower_val_access(eng.to_reg(num_idxs_reg))],
            outs=[_out_ap],
            transpose=False,
            num_idxs=num_idxs,
            elem_size=elem_size,
            stride_bytes_256=stride_bytes_256,
            gen_mode=0,
            single_packet=single_packet,
            queue_num=queue_num,
            sbuf_tokens_per_rank=0,
            sbuf_free_dim_per_rank=0,
            sbuf_free_dim_pad_per_rank=0,
            sbuf_byte_offset=0,
        )
    )


FP16 = mybir.dt.float16
FP32 = mybir.dt.float32
INT16 = mybir.dt.int16
Alu = mybir.AluOpType
Act = mybir.ActivationFunctionType

N_CORES = 8
MD = 64            # dst-block size
NQ = 4             # src subtables (slot % 4); NPAD/4 must fit int16
GPB = 7            # dst-blocks per group


class Cfg:
    def __init__(self, n_nodes, in_ch, hid, bcols):
        self.in_ch = in_ch
        self.hid = hid
        self.bcols = bcols
        self.n_cores = N_CORES
        npc = math.ceil(n_nodes / N_CORES)
        lcm = math.lcm(128, MD)
        npc = ((npc + lcm - 1) // lcm) * lcm
        self.npc = npc
        self.npad = npc * N_CORES
        assert self.npad % NQ == 0
        self.qsz = self.npad // NQ
        assert self.qsz <= 32768
        self.nb = npc // MD                    # dst blocks per core
        self.ngroups = math.ceil(self.nb / GPB)
        self.bcap = bcols * 128                # slots per bucket
        self.ncols = self.nb * NQ * bcols      # grid columns per core
        self.ntile128 = npc // 128
        self.ntile128f = self.npad // 128

    def group_blocks(self, g):
        b0 = g * GPB
        return b0, min(GPB, self.nb - b0)


def host_prep(x, edge_index, W1, b1, W2, b2, Wl1, bl1, Wl2, bl2):
    n = x.shape[0]
    in_ch = x.shape[1]
    hid = W1.shape[1]
    cfg = Cfg(n, in_ch, hid, 9)
    npc, npad, nb = cfg.npc, cfg.npad, cfg.nb

    loops = np.arange(n, dtype=np.int64)
    src = np.concatenate([np.asarray(edge_index[0], dtype=np.int64), loops])
    dst = np.concatenate([np.asarray(edge_index[1], dtype=np.int64), loops])
    srcq = (src % NQ).astype(np.int64)

    # node -> slot permutation. Start with identity (residue-preserving) and
    # swap same-residue nodes between blocks until no (block, quarter) bucket
    # exceeds the cap (8 columns; fall back to 9 if balancing stalls).
    slot_of = np.arange(npad, dtype=np.int64)       # node -> slot (pads tail)
    # per-node in-edge quarter profile
    prof = np.zeros((npad, NQ), dtype=np.int64)
    np.add.at(prof, (dst, srcq), 1)

    blk = slot_of[dst] // MD
    cnt = np.bincount(blk * NQ + srcq,
                      minlength=npad // MD * NQ).reshape(-1, NQ)
    node_of = np.arange(npad, dtype=np.int64)       # slot -> node
    cap = cfg.bcap
    for _round in range(40000):
        over_b, over_q = np.nonzero(cnt > cap)
        if len(over_b) == 0:
            break
        bo, q = over_b[0], over_q[0]
        # donor: node in block bo with largest q-profile
        nodes_bo = node_of[np.arange(bo * MD, (bo + 1) * MD)]
        dn = nodes_bo[np.argmax(prof[nodes_bo, q])]
        r = dn % NQ
        # recipient: among 32 lightest-in-q blocks, min worst-quarter load
        nb_all = cnt.shape[0]
        cand_b = np.argpartition(cnt[:, q], min(32, nb_all - 1))[:32]
        worst = (cnt[cand_b] + prof[dn]).max(axis=1)
        br = int(cand_b[np.argmin(worst)])
        if br == bo:
            cap = None
            break
        nodes_br = node_of[np.arange(br * MD, (br + 1) * MD)]
        cand = np.nonzero(nodes_br % NQ == r)[0]
        cn = nodes_br[cand[np.argmin(prof[nodes_br[cand], q])]]
        s1, s2 = slot_of[dn], slot_of[cn]
        slot_of[dn], slot_of[cn] = s2, s1
        node_of[s1], node_of[s2] = cn, dn
        cnt[bo] += prof[cn] - prof[dn]
        cnt[br] += prof[dn] - prof[cn]
    bcols = int(np.ceil(cnt.max() / 128))
    if bcols != cfg.bcols:
        cfg = Cfg(n, in_ch, hid, bcols)
    BCOLS = cfg.bcols
    assert (cnt <= cfg.bcap).all()

    sdst = slot_of[dst]
    ssrc = slot_of[src]
    gq = ssrc % NQ
    gidx_val = (ssrc // NQ).astype(np.int16)
    core = sdst // npc
    blk_l = (sdst % npc) // MD
    dloc_val = (sdst % MD).astype(np.float16)

    # order edges by (core, block, quarter); fill bucket slots sequentially
    key = (core * nb + blk_l) * NQ + gq
    order = np.argsort(key, kind="stable")
    key_s = key[order]
    gidx_s = gidx_val[order]
    dloc_s = dloc_val[order]
    cnts = np.bincount(key_s, minlength=N_CORES * nb * NQ)
    ofs = np.concatenate([[0], np.cumsum(cnts)])

    # column index of bucket (b, q) within its core's grid:
    #   group g = b // GPB; within group: q-major then block then col
    def bucket_col0(b, q):
        g = b // GPB
        b0, nbg = cfg.group_blocks(g)
        base = b0 * NQ * BCOLS
        return base + q * (nbg * BCOLS) + (b - b0) * BCOLS

    gidx_all, dstloc_all = [], []
    for c in range(N_CORES):
        gsl = np.zeros((128, cfg.ncols), dtype=np.int16)
        dloc = np.full((128, cfg.ncols), 10000.0, dtype=np.float16)
        for b in range(nb):
            for q in range(NQ):
                k = (c * nb + b) * NQ + q
                s, e = ofs[k], ofs[k + 1]
                if e == s:
                    continue
                col0 = bucket_col0(b, q)
                sl = np.arange(e - s)
                p_i, t_i = sl % 128, sl // 128
                gsl[p_i, col0 + t_i] = gidx_s[s:e]
                dloc[p_i, col0 + t_i] = dloc_s[s:e]
        # int16 idx stream: 16-partition wrapped, replicated x8
        flat = gsl.T.reshape(-1)
        w = flat.reshape(-1, 16)
        idxw = np.zeros((128, cfg.ncols * 8), dtype=np.int16)
        for g8 in range(8):
            idxw[g8 * 16:(g8 + 1) * 16, :] = w.T
        gidx_all.append(idxw)
        dstloc_all.append(dloc)

    # degree per slot (in-degree incl. self loop); pads get deg 1
    deg = np.zeros(npad, dtype=np.float32)
    np.add.at(deg, sdst, 1.0)
    deg[deg == 0] = 1.0
    dinv = 1.0 / np.sqrt(deg)

    xf = np.zeros((npad, in_ch), dtype=np.float32)
    xf[slot_of[:n]] = np.asarray(x, dtype=np.float32)

    dinvT_all, dinv128_all, xT_all = [], [], []
    for c in range(N_CORES):
        d = dinv[c * npc:(c + 1) * npc]
        dinvT_all.append(np.tile(d.astype(np.float16)[None, :], (hid, 1)))
        dinv128_all.append(np.ascontiguousarray(
            d.reshape(cfg.ntile128, 128).T).astype(np.float32))
        xT_all.append(np.ascontiguousarray(
            xf[c * npc:(c + 1) * npc].T).astype(np.float16))

    iota = np.tile(np.arange(MD, dtype=np.float16)[None, :], (128, 1))

    consts = {
        "W1": np.asarray(W1, np.float16),
        "W2": np.asarray(W2, np.float16),
        "Wl1": np.asarray(Wl1, np.float16),
        "Wl2": np.asarray(Wl2, np.float16),
        "b1c": np.asarray(b1, np.float32).reshape(hid, 1),
        "b2c": np.asarray(b2, np.float32).reshape(hid, 1),
        "bl1": np.asarray(bl1, np.float32).reshape(hid, 1),
        "bl2": np.asarray(bl2, np.float32).reshape(1, 1),
        "iotaM": iota,
    }
    in_maps = []
    for c in range(N_CORES):
        m = dict(consts)
        m["xT"] = xT_all[c]
        m["gidx"] = gidx_all[c]
        m["dstloc"] = dstloc_all[c]
        m["dinvT"] = dinvT_all[c]
        m["dinv128"] = dinv128_all[c]
        in_maps.append(m)
    return cfg, in_maps, node_of


def build_program(cfg: Cfg):
    nc = bacc.Bacc("TRN2", target_bir_lowering=False, num_swdge_queues=4)
    H, NPC, NB = cfg.hid, cfg.npc, cfg.nb
    NPAD, BCOLS = cfg.npad, cfg.bcols
    GCMAX = GPB * NQ * BCOLS

    xT_d = nc.dram_tensor("xT", [cfg.in_ch, NPC], FP16, kind="ExternalInput")
    W1_d = nc.dram_tensor("W1", [cfg.in_ch, H], FP16, kind="ExternalInput")
    W2_d = nc.dram_tensor("W2", [H, H], FP16, kind="ExternalInput")
    Wl1_d = nc.dram_tensor("Wl1", [H, H], FP16, kind="ExternalInput")
    Wl2_d = nc.dram_tensor("Wl2", [H, 1], FP16, kind="ExternalInput")
    b1c_d = nc.dram_tensor("b1c", [H, 1], FP32, kind="ExternalInput")
    b2c_d = nc.dram_tensor("b2c", [H, 1], FP32, kind="ExternalInput")
    bl1_d = nc.dram_tensor("bl1", [H, 1], FP32, kind="ExternalInput")
    bl2_d = nc.dram_tensor("bl2", [1, 1], FP32, kind="ExternalInput")
    dinvT_d = nc.dram_tensor("dinvT", [H, NPC], FP16, kind="ExternalInput")
    dinv128_d = nc.dram_tensor("dinv128", [128, cfg.ntile128], FP32,
                               kind="ExternalInput")
    gidx_d = nc.dram_tensor("gidx", [128, cfg.ncols * 8], INT16,
                            kind="ExternalInput")
    dstloc_d = nc.dram_tensor("dstloc", [128, cfg.ncols], FP16,
                              kind="ExternalInput")
    iota_d = nc.dram_tensor("iotaM", [128, MD], FP16, kind="ExternalInput")
    y_d = nc.dram_tensor("y", [NPC], FP32, kind="ExternalOutput")

    t1s_d = nc.dram_tensor("t1s", [NPC, H], FP16)
    t2s_d = nc.dram_tensor("t2s", [NPC, H], FP16)
    t1c_d = nc.dram_tensor("t1c", [NPAD, H], FP16, addr_space="Shared")
    t2c_d = nc.dram_tensor("t2c", [NPAD, H], FP16, addr_space="Shared")

    dstloc_s = nc.alloc_sbuf_tensor("dstloc_s", [128, cfg.ncols], FP16).ap()
    iota_s = nc.alloc_sbuf_tensor("iota_s", [128, MD], FP16).ap()
    W2_s = nc.alloc_sbuf_tensor("W2_s", [H, H], FP16).ap()
    Wl1_s = nc.alloc_sbuf_tensor("Wl1_s", [H, H], FP16).ap()
    Wl2_s = nc.alloc_sbuf_tensor("Wl2_s", [H, 1], FP16).ap()
    b1c_s = nc.alloc_sbuf_tensor("b1c_s", [H, 1], FP32).ap()
    b2c_s = nc.alloc_sbuf_tensor("b2c_s", [H, 1], FP32).ap()
    bl1_s = nc.alloc_sbuf_tensor("bl1_s", [H, 1], FP32).ap()
    bl2_s = nc.alloc_sbuf_tensor("bl2_s", [1, 1], FP32).ap()
    dinvT_s = nc.alloc_sbuf_tensor("dinvT_s", [H, NPC], FP16).ap()
    dinv128_s = nc.alloc_sbuf_tensor("dinv128_s", [128, cfg.ntile128],
                                     FP32).ap()

    cc_sem = nc.alloc_semaphore("cc_sem")

    # subtable views: rows q::4 of t_c -> stride NQ*H elems (256B), offset q*H
    def subtable(tc_d, q):
        return tc_d[:].rearrange("(i q) e -> i (q e)", q=NQ)[
            :, q * H:(q + 1) * H]

    def make_table_tiles(pool, psum, hT, W_s, ts_d, j0, j1):
        """T[j0*128:(j1)*128] = fp16(dinv * (h @ W)) -> ts_d rows."""
        for ja in range(j0, j1, 4):
            jb = min(ja + 4, j1)
            k = jb - ja
            ps = psum.tile([128, 4, H], FP32, tag="tbl_ps")
            for j in range(ja, jb):
                nc.tensor.matmul(ps[:, j - ja, :],
                                 hT[:, j * 128:(j + 1) * 128], W_s[:],
                                 start=True, stop=True)
            ts = pool.tile([128, 4, H], FP16, tag="tbl_sb")
            dv = dinv128_s[:, ja:jb].rearrange("p (j a) -> p j a", a=1)
            nc.vector.tensor_tensor(ts[:, :k, :], ps[:, :k, :],
                                    dv.to_broadcast([128, k, H]), Alu.mult)
            nc.sync.dma_start(
                ts_d[ja * 128:jb * 128, :].rearrange(
                    "(j p) e -> p j e", p=128), ts[:, :k, :])

    def agg_layer(gpool, bigpool, pool, psum, tc_d, bc_s, hT_out, layer):
        """Aggregate one GCN layer group-by-group; fused tails."""
        wm = 0  # T2 watermark in 128-node tiles (layer 1 only)
        for g in range(cfg.ngroups):
            b0, nbg = cfg.group_blocks(g)
            gc = nbg * NQ * BCOLS            # grid cols this group
            col0 = b0 * NQ * BCOLS
            qcols = nbg * BCOLS              # cols per quarter
            nodes0 = b0 * MD
            gn = nbg * MD                    # nodes this group

            idxb = gpool.tile([128, GPB * NQ * BCOLS * 8], INT16, tag="idx")
            nc.sync.dma_start(idxb[:, :gc * 8],
                              gidx_d[:, col0 * 8:(col0 + gc) * 8])
            msgs = []
            for q in range(NQ):
                mt = gpool.tile([128, GPB * BCOLS, H], FP16, tag=f"msg{q}")
                msgs.append(mt)
                dma_gather_raw(
                    nc.gpsimd, mt[:, :qcols, :], subtable(tc_d, q),
                    idxb[:, q * qcols * 8:(q + 1) * qcols * 8],
                    qcols * 128, qcols * 128, H, NQ * H,
                    queue_num=(g * NQ + q) % 4)

            # scatter one-hot, column-major: oh[p, c, d] = (dstloc[p,c] == d)
            oh = bigpool.tile([128, GPB * NQ * BCOLS, MD], FP16, tag="oh")
            dl = dstloc_s[:, col0:col0 + gc]
            dl3 = dl.rearrange("p (t a) -> p t a", a=1).to_broadcast(
                [128, gc, MD])
            io3 = iota_s[:].rearrange("p (a d) -> p a d", a=1).to_broadcast(
                [128, gc, MD])
            nc.vector.tensor_tensor(oh[:, :gc, :], dl3, io3, Alu.is_equal)

            # out[f, d] += msgs[s, f]^T @ oh[s, d], accumulated per block
            ps = psum.tile([H, GPB, MD], FP32, tag="agg_ps")
            for bl in range(nbg):
                im = 0
                nmm = NQ * BCOLS
                for q in range(NQ):
                    for t in range(BCOLS):
                        ct = q * qcols + bl * BCOLS + t
                        nc.tensor.matmul(ps[:, bl, :],
                                         msgs[q][:, bl * BCOLS + t, :],
                                         oh[:, ct, :],
                                         start=(im == 0), stop=(im == nmm - 1))
                        im += 1

            # tail: hT = relu(dinv_d * sum + b)  (self loop is in the grid)
            tsum = pool.tile([H, GPB * MD], FP32, tag="tsum")
            nc.vector.tensor_tensor(
                tsum[:, :gn],
                ps[:].rearrange("f b d -> f (b d)")[:, :gn],
                dinvT_s[:, nodes0:nodes0 + gn], Alu.mult)
            nc.scalar.activation(hT_out[:, nodes0:nodes0 + gn],
                                 tsum[:, :gn], Act.Relu, bias=bc_s[:, 0:1])

            if layer == 1:
                # T2 for completed 128-node tiles
                hi = (nodes0 + gn) // 128
                if hi > wm:
                    make_table_tiles(pool, psum, hT_out, W2_s, t2s_d, wm, hi)
                    wm = hi
            else:
                # MLP tail for this group's nodes
                zp = psum.tile([H, GPB * MD], FP32, tag="zps")
                nc.tensor.matmul(zp[:, :gn], Wl1_s[:],
                                 hT_out[:, nodes0:nodes0 + gn],
                                 start=True, stop=True)
                zt = pool.tile([H, GPB * MD], FP16, tag="zt")
                nc.scalar.activation(zt[:, :gn], zp[:, :gn], Act.Relu,
                                     bias=bl1_s[:, 0:1])
                yp = psum.tile([1, GPB * MD], FP32, tag="yps")
                nc.tensor.matmul(yp[:, :gn], Wl2_s[:], zt[:, :gn],
                                 start=True, stop=True)
                ys = pool.tile([1, GPB * MD], FP32, tag="ysl")
                nc.scalar.activation(ys[:, :gn], yp[:, :gn], Act.Identity,
                                     bias=bl2_s[:, 0:1])
                y2 = y_d[:].rearrange("(a n) -> a n", a=1)
                nc.sync.dma_start(y2[:, nodes0:nodes0 + gn], ys[:, :gn])

    # ------- Phase 1: constants; every core builds the FULL T1 table -------
    with tile.TileContext(nc) as tc:
        with tc.tile_pool(name="p1", bufs=2) as pool, \
             tc.tile_pool(name="p1ps", bufs=2, space="PSUM") as psum:
            nc.sync.dma_start(dstloc_s[:], dstloc_d[:])
            nc.sync.dma_start(iota_s[:], iota_d[:])
            nc.sync.dma_start(W2_s[:], W2_d[:])
            nc.sync.dma_start(Wl1_s[:], Wl1_d[:])
            nc.sync.dma_start(Wl2_s[:], Wl2_d[:])
            nc.sync.dma_start(b1c_s[:], b1c_d[:])
            nc.sync.dma_start(b2c_s[:], b2c_d[:])
            nc.sync.dma_start(bl1_s[:], bl1_d[:])
            nc.sync.dma_start(bl2_s[:], bl2_d[:])

            nc.sync.dma_start(dinvT_s[:], dinvT_d[:])
            nc.sync.dma_start(dinv128_s[:], dinv128_d[:])

            xT = pool.tile([cfg.in_ch, NPC], FP16)
            nc.sync.dma_start(xT[:], xT_d[:])
            W1 = pool.tile([cfg.in_ch, H], FP16)
            nc.sync.dma_start(W1[:], W1_d[:])
            make_table_tiles(pool, psum, xT, W1, t1s_d, 0, cfg.ntile128)

    nc.gpsimd.collective_compute(
        "AllGather", Alu.bypass, replica_groups=[list(range(N_CORES))],
        ins=[t1s_d[:]], outs=[t1c_d[:]]).then_inc(cc_sem, 1)
    nc.gpsimd.wait_ge(cc_sem, 1)

    # ---------------- Phase 2: aggregate L1 (+T2 build) ----------------
    with tile.TileContext(nc) as tc:
        with tc.tile_pool(name="p2b", bufs=2) as bigpool, \
             tc.tile_pool(name="p2g", bufs=2) as gpool, \
             tc.tile_pool(name="p2", bufs=3) as pool, \
             tc.tile_pool(name="p2h", bufs=1) as hpool, \
             tc.tile_pool(name="p2ps", bufs=2, space="PSUM") as psum:
            h1T = hpool.tile([H, NPC], FP16)
            agg_layer(gpool, bigpool, pool, psum, t1c_d, b1c_s, h1T, layer=1)

    nc.gpsimd.collective_compute(
        "AllGather", Alu.bypass, replica_groups=[list(range(N_CORES))],
        ins=[t2s_d[:]], outs=[t2c_d[:]]).then_inc(cc_sem, 1)
    nc.gpsimd.wait_ge(cc_sem, 2)

    # ---------------- Phase 3: aggregate L2 + MLP ----------------
    with tile.TileContext(nc) as tc:
        with tc.tile_pool(name="p3b", bufs=2) as bigpool, \
             tc.tile_pool(name="p3g", bufs=2) as gpool, \
             tc.tile_pool(name="p3", bufs=3) as pool, \
             tc.tile_pool(name="p3h", bufs=1) as hpool, \
             tc.tile_pool(name="p3ps", bufs=2, space="PSUM") as psum:
            h2T = hpool.tile([H, NPC], FP16)
            agg_layer(gpool, bigpool, pool, psum, t2c_d, b2c_s, h2T, layer=2)

    nc.compile()
    return nc


_CACHE = {}


def _get_program(key, cfg):
    if key not in _CACHE:
        _CACHE[key] = build_program(cfg)
    return _CACHE[key]


def kernel(x, edge_index, W1, b1, W2, b2, Wl1, bl1, Wl2, bl2):
    x = np.asarray(x)
    n = x.shape[0]
    cfg, in_maps, node_of = host_prep(x, edge_index, W1, b1, W2, b2,
                                      Wl1, bl1, Wl2, bl2)
    key = (n, cfg.in_ch, cfg.hid, cfg.bcols)
    nc = _get_program(key, cfg)
    res = run_bass_kernel_spmd(nc, in_maps, list(range(N_CORES)))
    ys = [res.results[c]["y"].reshape(-1) for c in range(N_CORES)]
    y_slots = np.concatenate(ys)
    # slot s holds output of node node_of[s]
    y = np.empty(n, dtype=np.float32)
    valid = node_of < n
    y[node_of[valid]] = y_slots[valid]
    return y.reshape(n, 1)


# revision 36
# speedup vs baseline: 1.6989x; 1.0798x over previous
"""Trainium2 Bass kernel for a 2-layer GCN + 2-layer MLP (gnn_message_passing).

Model (see reference):
    h1 = relu(GCNConv(x;  W1, b1))       # symmetric-normalized, self-loops
    h2 = relu(GCNConv(h1; W2, b2))
    h3 = relu(h2 @ Wl1 + bl1)
    y  = h3 @ Wl2 + bl2                  # [N, 1]

Distribution: nodes are RELABELED by a host-chosen permutation into 8 shards
of NPC slots; each core aggregates the edges whose destination it owns.  Per
layer each core computes the scaled table T = dinv * (h @ W) for its shard,
the shards are AllGathered into t_c [NPAD, 32] fp16, and messages T[src] are
fetched per edge with SWDGE dma_gather.

Key layout tricks vs. a naive port:
  * The compact table IS 4 interleaved 256B-strided subtables: rows q::4 have
    a 256-byte stride, so gathers read t_c directly (no strided "expand"
    copy); an edge's subtable is slot(src) % 4 and its index slot(src) // 4,
    which fits int16 because NPAD/4 < 32768.
  * Edge slot grid: buckets keyed by (dst-block of 64, src%4) with a uniform
    capacity of BCAP slots (BCOLS columns of 128).  The host rebalances the
    node->slot permutation (swapping equal-residue nodes between blocks) so
    no bucket overflows.  Self-loop edges live in the grid like any edge.
  * The scatter matmul runs "flipped": lhsT = the gathered messages column
    (contiguous [128, 32] weights), rhs = the one-hot column (contiguous
    [128, 64]), so the output lands feat-major [32, 64] in PSUM and no
    transposes are needed; one PSUM tile [32, GPB, 64] holds a whole group.
  * Tail = one DVE multiply by a replicated dinv row + one ScalarE
    bias+relu activation straight into hT.  The T2 table build (layer 1)
    and the final MLP (layer 2) run per-group, interleaved with gathers.
  * Per-descriptor cost of SWDGE gather is ~2.4 ns regardless of payload
    (HW-measured); total slots is the wall, so padding is minimized.
"""

import math
import sys

import numpy as np

sys.path.insert(0, "/opt/trn_rl_repo")
sys.path.insert(0, "/root/problem")

import concourse.bass as bass
import concourse.mybir as mybir
import concourse.ap_utils as ap_utils
import concourse.tile as tile
from concourse import bacc
from concourse._compat import exact_div
from concourse.bass_utils import run_bass_kernel_spmd


def dma_gather_raw(eng, out_ap, in_ap, idxs_ap, num_idxs, num_idxs_reg,
                   elem_size, elem_step, single_packet=False, queue_num=0):
    """gathered = in[idxs, :elem_size]; rows strided elem_step elements.

    Clone of BassGpSimd.dma_gather's HBM path minus the
    `elem_size_bytes % 256 == 0` restriction (the Q7 ucode only requires the
    row STRIDE to be a 256-byte multiple; payload bytes are free)."""
    assert idxs_ap.dtype == mybir.dt.int16
    assert in_ap.dtype == out_ap.dtype
    dt_size = mybir.dt.size(in_ap.dtype)
    assert ap_utils.ap_is_contiguous(out_ap.ap[1:])
    assert ap_utils.ap_is_contiguous(idxs_ap.ap[1:])
    assert in_ap.ap[-1][1] == out_ap.ap[-1][1] == elem_size
    assert out_ap.ap[0][1] * out_ap.ap[1][1] == ((num_idxs + 127) // 128) * 128
    assert in_ap.ap[0][0] == elem_step
    stride_bytes_256 = exact_div(elem_step * dt_size, 256)
    assert stride_bytes_256 < 256

    _in_ap = eng.lower_ap_dma(in_ap, for_custom_bir_dma=True)
    _idxs_ap = eng.lower_ap(idxs_ap)
    _out_ap = eng.lower_ap(out_ap)
    return eng.add_instruction(
        mybir.InstDMAGatherAnt(
            name=eng.bass.get_next_instruction_name(),
            ins=[*_in_ap, _idxs_ap,
                 eng.lower_val_access(eng.to_reg(num_idxs_reg))],
            outs=[_out_ap],
            transpose=False,
            num_idxs=num_idxs,
            elem_size=elem_size,
            stride_bytes_256=stride_bytes_256,
            gen_mode=0,
            single_packet=single_packet,
            queue_num=queue_num,
            sbuf_tokens_per_rank=0,
            sbuf_free_dim_per_rank=0,
            sbuf_free_dim_pad_per_rank=0,
            sbuf_byte_offset=0,
        )
    )


FP16 = mybir.dt.float16
FP32 = mybir.dt.float32
INT16 = mybir.dt.int16
Alu = mybir.AluOpType
Act = mybir.ActivationFunctionType

N_CORES = 8
MD = 64            # dst-block size
NQ = 4             # src subtables (slot % 4); NPAD/4 must fit int16
GPB = 7            # dst-blocks per group


class Cfg:
    def __init__(self, n_nodes, in_ch, hid, bcols):
        self.in_ch = in_ch
        self.hid = hid
        self.bcols = bcols
        self.n_cores = N_CORES
        npc = math.ceil(n_nodes / N_CORES)
        lcm = math.lcm(128, MD)
        npc = ((npc + lcm - 1) // lcm) * lcm
        self.npc = npc
        self.npad = npc * N_CORES
        assert self.npad % NQ == 0
        self.qsz = self.npad // NQ
        assert self.qsz <= 32768
        self.nb = npc // MD                    # dst blocks per core
        self.ngroups = math.ceil(self.nb / GPB)
        self.bcap = bcols * 128                # slots per bucket
        self.ncols = self.nb * NQ * bcols      # grid columns per core
        self.ntile128 = npc // 128
        self.ntile128f = self.npad // 128

    def group_blocks(self, g):
        b0 = g * GPB
        return b0, min(GPB, self.nb - b0)


def host_prep(x, edge_index, W1, b1, W2, b2, Wl1, bl1, Wl2, bl2):
    n = x.shape[0]
    in_ch = x.shape[1]
    hid = W1.shape[1]
    cfg = Cfg(n, in_ch, hid, 9)
    npc, npad, nb = cfg.npc, cfg.npad, cfg.nb

    loops = np.arange(n, dtype=np.int64)
    src = np.concatenate([np.asarray(edge_index[0], dtype=np.int64), loops])
    dst = np.concatenate([np.asarray(edge_index[1], dtype=np.int64), loops])
    srcq = (src % NQ).astype(np.int64)

    # node -> slot permutation. Start with identity (residue-preserving) and
    # swap same-residue nodes between blocks until no (block, quarter) bucket
    # exceeds the cap (8 columns; fall back to 9 if balancing stalls).
    slot_of = np.arange(npad, dtype=np.int64)       # node -> slot (pads tail)
    # per-node in-edge quarter profile
    prof = np.zeros((npad, NQ), dtype=np.int64)
    np.add.at(prof, (dst, srcq), 1)

    blk = slot_of[dst] // MD
    cnt = np.bincount(blk * NQ + srcq,
                      minlength=npad // MD * NQ).reshape(-1, NQ)
    node_of = np.arange(npad, dtype=np.int64)       # slot -> node
    cap = cfg.bcap
    for _round in range(40000):
        over_b, over_q = np.nonzero(cnt > cap)
        if len(over_b) == 0:
            break
        bo, q = over_b[0], over_q[0]
        # donor: node in block bo with largest q-profile
        nodes_bo = node_of[np.arange(bo * MD, (bo + 1) * MD)]
        dn = nodes_bo[np.argmax(prof[nodes_bo, q])]
        r = dn % NQ
        # recipient: among 32 lightest-in-q blocks, min worst-quarter load
        nb_all = cnt.shape[0]
        cand_b = np.argpartition(cnt[:, q], min(32, nb_all - 1))[:32]
        worst = (cnt[cand_b] + prof[dn]).max(axis=1)
        br = int(cand_b[np.argmin(worst)])
        if br == bo:
            cap = None
            break
        nodes_br = node_of[np.arange(br * MD, (br + 1) * MD)]
        cand = np.nonzero(nodes_br % NQ == r)[0]
        cn = nodes_br[cand[np.argmin(prof[nodes_br[cand], q])]]
        s1, s2 = slot_of[dn], slot_of[cn]
        slot_of[dn], slot_of[cn] = s2, s1
        node_of[s1], node_of[s2] = cn, dn
        cnt[bo] += prof[cn] - prof[dn]
        cnt[br] += prof[dn] - prof[cn]
    bcols = int(np.ceil(cnt.max() / 128))
    if bcols != cfg.bcols:
        cfg = Cfg(n, in_ch, hid, bcols)
    BCOLS = cfg.bcols
    assert (cnt <= cfg.bcap).all()

    sdst = slot_of[dst]
    ssrc = slot_of[src]
    gq = ssrc % NQ
    gidx_val = (ssrc // NQ).astype(np.int16)
    core = sdst // npc
    blk_l = (sdst % npc) // MD
    dloc_val = (sdst % MD).astype(np.float16)

    # order edges by (core, block, quarter); fill bucket slots sequentially
    key = (core * nb + blk_l) * NQ + gq
    order = np.argsort(key, kind="stable")
    key_s = key[order]
    gidx_s = gidx_val[order]
    dloc_s = dloc_val[order]
    cnts = np.bincount(key_s, minlength=N_CORES * nb * NQ)
    ofs = np.concatenate([[0], np.cumsum(cnts)])

    # column index of bucket (b, q) within its core's grid:
    #   group g = b // GPB; within group: q-major then block then col
    def bucket_col0(b, q):
        g = b // GPB
        b0, nbg = cfg.group_blocks(g)
        base = b0 * NQ * BCOLS
        return base + q * (nbg * BCOLS) + (b - b0) * BCOLS

    gidx_all, dstloc_all = [], []
    for c in range(N_CORES):
        gsl = np.zeros((128, cfg.ncols), dtype=np.int16)
        dloc = np.full((128, cfg.ncols), 10000.0, dtype=np.float16)
        for b in range(nb):
            for q in range(NQ):
                k = (c * nb + b) * NQ + q
                s, e = ofs[k], ofs[k + 1]
                if e == s:
                    continue
                col0 = bucket_col0(b, q)
                sl = np.arange(e - s)
                p_i, t_i = sl % 128, sl // 128
                gsl[p_i, col0 + t_i] = gidx_s[s:e]
                dloc[p_i, col0 + t_i] = dloc_s[s:e]
        # int16 idx stream: 16-partition wrapped, replicated x8
        flat = gsl.T.reshape(-1)
        w = flat.reshape(-1, 16)
        idxw = np.zeros((128, cfg.ncols * 8), dtype=np.int16)
        for g8 in range(8):
            idxw[g8 * 16:(g8 + 1) * 16, :] = w.T
        gidx_all.append(idxw)
        dstloc_all.append(dloc)

    # degree per slot (in-degree incl. self loop); pads get deg 1
    deg = np.zeros(npad, dtype=np.float32)
    np.add.at(deg, sdst, 1.0)
    deg[deg == 0] = 1.0
    dinv = 1.0 / np.sqrt(deg)

    xf = np.zeros((npad, in_ch), dtype=np.float32)
    xf[slot_of[:n]] = np.asarray(x, dtype=np.float32)

    dinvT_all, dinv128_all, xT_all = [], [], []
    for c in range(N_CORES):
        d = dinv[c * npc:(c + 1) * npc]
        dinvT_all.append(np.tile(d.astype(np.float16)[None, :], (hid, 1)))
        dinv128_all.append(np.ascontiguousarray(
            d.reshape(cfg.ntile128, 128).T).astype(np.float32))
        xT_all.append(np.ascontiguousarray(
            xf[c * npc:(c + 1) * npc].T).astype(np.float16))

    iota = np.tile(np.arange(MD, dtype=np.float16)[None, :], (128, 1))

    consts = {
        "W1": np.asarray(W1, np.float16),
        "W2": np.asarray(W2, np.float16),
        "Wl1": np.asarray(Wl1, np.float16),
        "Wl2": np.asarray(Wl2, np.float16),
        "b1c": np.asarray(b1, np.float32).reshape(hid, 1),
        "b2c": np.asarray(b2, np.float32).reshape(hid, 1),
        "bl1": np.asarray(bl1, np.float32).reshape(hid, 1),
        "bl2": np.asarray(bl2, np.float32).reshape(1, 1),
        "iotaM": iota,
    }
    in_maps = []
    for c in range(N_CORES):
        m = dict(consts)
        m["xT"] = xT_all[c]
        m["gidx"] = gidx_all[c]
        m["dstloc"] = dstloc_all[c]
        m["dinvT"] = dinvT_all[c]
        m["dinv128"] = dinv128_all[c]
        in_maps.append(m)
    return cfg, in_maps, node_of


def build_program(cfg: Cfg):
    nc = bacc.Bacc("TRN2", target_bir_lowering=False, num_swdge_queues=4)
    H, NPC, NB = cfg.hid, cfg.npc, cfg.nb
    NPAD, BCOLS = cfg.npad, cfg.bcols
    GCMAX = GPB * NQ * BCOLS

    xT_d = nc.dram_tensor("xT", [cfg.in_ch, NPC], FP16, kind="ExternalInput")
    W1_d = nc.dram_tensor("W1", [cfg.in_ch, H], FP16, kind="ExternalInput")
    W2_d = nc.dram_tensor("W2", [H, H], FP16, kind="ExternalInput")
    Wl1_d = nc.dram_tensor("Wl1", [H, H], FP16, kind="ExternalInput")
    Wl2_d = nc.dram_tensor("Wl2", [H, 1], FP16, kind="ExternalInput")
    b1c_d = nc.dram_tensor("b1c", [H, 1], FP32, kind="ExternalInput")
    b2c_d = nc.dram_tensor("b2c", [H, 1], FP32, kind="ExternalInput")
    bl1_d = nc.dram_tensor("bl1", [H, 1], FP32, kind="ExternalInput")
    bl2_d = nc.dram_tensor("bl2", [1, 1], FP32, kind="ExternalInput")
    dinvT_d = nc.dram_tensor("dinvT", [H, NPC], FP16, kind="ExternalInput")
    dinv128_d = nc.dram_tensor("dinv128", [128, cfg.ntile128], FP32,
                               kind="ExternalInput")
    gidx_d = nc.dram_tensor("gidx", [128, cfg.ncols * 8], INT16,
                            kind="ExternalInput")
    dstloc_d = nc.dram_tensor("dstloc", [128, cfg.ncols], FP16,
                              kind="ExternalInput")
    iota_d = nc.dram_tensor("iotaM", [128, MD], FP16, kind="ExternalInput")
    y_d = nc.dram_tensor("y", [NPC], FP32, kind="ExternalOutput")

    t1s_d = nc.dram_tensor("t1s", [NPC, H], FP16)
    t2s_d = nc.dram_tensor("t2s", [NPC, H], FP16)
    t1c_d = nc.dram_tensor("t1c", [NPAD, H], FP16, addr_space="Shared")
    t2c_d = nc.dram_tensor("t2c", [NPAD, H], FP16, addr_space="Shared")

    dstloc_s = nc.alloc_sbuf_tensor("dstloc_s", [128, cfg.ncols], FP16).ap()
    iota_s = nc.alloc_sbuf_tensor("iota_s", [128, MD], FP16).ap()
    W2_s = nc.alloc_sbuf_tensor("W2_s", [H, H], FP16).ap()
    Wl1_s = nc.alloc_sbuf_tensor("Wl1_s", [H, H], FP16).ap()
    Wl2_s = nc.alloc_sbuf_tensor("Wl2_s", [H, 1], FP16).ap()
    b1c_s = nc.alloc_sbuf_tensor("b1c_s", [H, 1], FP32).ap()
    b2c_s = nc.alloc_sbuf_tensor("b2c_s", [H, 1], FP32).ap()
    bl1_s = nc.alloc_sbuf_tensor("bl1_s", [H, 1], FP32).ap()
    bl2_s = nc.alloc_sbuf_tensor("bl2_s", [1, 1], FP32).ap()
    dinv128_s = nc.alloc_sbuf_tensor("dinv128_s", [128, cfg.ntile128],
                                     FP32).ap()

    cc_sem = nc.alloc_semaphore("cc_sem")

    # subtable views: rows q::4 of t_c -> stride NQ*H elems (256B), offset q*H
    def subtable(tc_d, q):
        return tc_d[:].rearrange("(i q) e -> i (q e)", q=NQ)[
            :, q * H:(q + 1) * H]

    def make_table_tiles(pool, psum, hT, W_s, ts_d, j0, j1):
        """T[j0*128:(j1)*128] = fp16(dinv * (h @ W)) -> ts_d rows."""
        for ja in range(j0, j1, 4):
            jb = min(ja + 4, j1)
            k = jb - ja
            ps = psum.tile([128, 4, H], FP32, tag="tbl_ps")
            for j in range(ja, jb):
                nc.tensor.matmul(ps[:, j - ja, :],
                                 hT[:, j * 128:(j + 1) * 128], W_s[:],
                                 start=True, stop=True)
            ts = pool.tile([128, 4, H], FP16, tag="tbl_sb")
            dv = dinv128_s[:, ja:jb].rearrange("p (j a) -> p j a", a=1)
            nc.vector.tensor_tensor(ts[:, :k, :], ps[:, :k, :],
                                    dv.to_broadcast([128, k, H]), Alu.mult)
            nc.sync.dma_start(
                ts_d[ja * 128:jb * 128, :].rearrange(
                    "(j p) e -> p j e", p=128), ts[:, :k, :])

    def agg_layer(gpool, bigpool, pool, psum, tc_d, bc_s, hT_out, layer):
        """Aggregate one GCN layer group-by-group; fused tails."""
        wm = 0  # T2 watermark in 128-node tiles (layer 1 only)
        for g in range(cfg.ngroups):
            b0, nbg = cfg.group_blocks(g)
            gc = nbg * NQ * BCOLS            # grid cols this group
            col0 = b0 * NQ * BCOLS
            qcols = nbg * BCOLS              # cols per quarter
            nodes0 = b0 * MD
            gn = nbg * MD                    # nodes this group

            idxb = gpool.tile([128, GPB * NQ * BCOLS * 8], INT16, tag="idx")
            nc.sync.dma_start(idxb[:, :gc * 8],
                              gidx_d[:, col0 * 8:(col0 + gc) * 8])
            msgs = []
            for q in range(NQ):
                mt = gpool.tile([128, GPB * BCOLS, H], FP16, tag=f"msg{q}")
                msgs.append(mt)
                dma_gather_raw(
                    nc.gpsimd, mt[:, :qcols, :], subtable(tc_d, q),
                    idxb[:, q * qcols * 8:(q + 1) * qcols * 8],
                    qcols * 128, qcols * 128, H, NQ * H,
                    queue_num=(g * NQ + q) % 4)

            # scatter one-hot, column-major: oh[p, c, d] = (dstloc[p,c] == d)
            oh = bigpool.tile([128, GPB * NQ * BCOLS, MD], FP16, tag="oh")
            dl = dstloc_s[:, col0:col0 + gc]
            dl3 = dl.rearrange("p (t a) -> p t a", a=1).to_broadcast(
                [128, gc, MD])
            io3 = iota_s[:].rearrange("p (a d) -> p a d", a=1).to_broadcast(
                [128, gc, MD])
            nc.vector.tensor_tensor(oh[:, :gc, :], dl3, io3, Alu.is_equal)

            # out[f, d] += msgs[s, f]^T @ oh[s, d], accumulated per block
            ps = psum.tile([H, GPB, MD], FP32, tag="agg_ps")
            for bl in range(nbg):
                im = 0
                nmm = NQ * BCOLS
                for q in range(NQ):
                    for t in range(BCOLS):
                        ct = q * qcols + bl * BCOLS + t
                        nc.tensor.matmul(ps[:, bl, :],
                                         msgs[q][:, bl * BCOLS + t, :],
                                         oh[:, ct, :],
                                         start=(im == 0), stop=(im == nmm - 1))
                        im += 1

            # tail: hT = relu(dinv_d * sum + b)  (self loop is in the grid)
            dvt = pool.tile([H, GPB * MD], FP16, tag="dvt")
            nc.sync.dma_start(dvt[:, :gn], dinvT_d[:, nodes0:nodes0 + gn])
            tsum = pool.tile([H, GPB * MD], FP32, tag="tsum")
            nc.vector.tensor_tensor(
                tsum[:, :gn],
                ps[:].rearrange("f b d -> f (b d)")[:, :gn],
                dvt[:, :gn], Alu.mult)
            nc.scalar.activation(hT_out[:, nodes0:nodes0 + gn],
                                 tsum[:, :gn], Act.Relu, bias=bc_s[:, 0:1])

            if layer == 1:
                # T2 for completed 128-node tiles
                hi = (nodes0 + gn) // 128
                if hi > wm:
                    make_table_tiles(pool, psum, hT_out, W2_s, t2s_d, wm, hi)
                    wm = hi
            else:
                # MLP tail for this group's nodes
                zp = psum.tile([H, GPB * MD], FP32, tag="zps")
                nc.tensor.matmul(zp[:, :gn], Wl1_s[:],
                                 hT_out[:, nodes0:nodes0 + gn],
                                 start=True, stop=True)
                zt = pool.tile([H, GPB * MD], FP16, tag="zt")
                nc.scalar.activation(zt[:, :gn], zp[:, :gn], Act.Relu,
                                     bias=bl1_s[:, 0:1])
                yp = psum.tile([1, GPB * MD], FP32, tag="yps")
                nc.tensor.matmul(yp[:, :gn], Wl2_s[:], zt[:, :gn],
                                 start=True, stop=True)
                ys = pool.tile([1, GPB * MD], FP32, tag="ysl")
                nc.scalar.activation(ys[:, :gn], yp[:, :gn], Act.Identity,
                                     bias=bl2_s[:, 0:1])
                y2 = y_d[:].rearrange("(a n) -> a n", a=1)
                nc.sync.dma_start(y2[:, nodes0:nodes0 + gn], ys[:, :gn])

    # ------- Phase 1: constants; every core builds the FULL T1 table -------
    with tile.TileContext(nc) as tc:
        with tc.tile_pool(name="p1", bufs=2) as pool, \
             tc.tile_pool(name="p1ps", bufs=2, space="PSUM") as psum:
            nc.sync.dma_start(dstloc_s[:], dstloc_d[:])
            nc.sync.dma_start(iota_s[:], iota_d[:])
            nc.sync.dma_start(W2_s[:], W2_d[:])
            nc.sync.dma_start(Wl1_s[:], Wl1_d[:])
            nc.sync.dma_start(Wl2_s[:], Wl2_d[:])
            nc.sync.dma_start(b1c_s[:], b1c_d[:])
            nc.sync.dma_start(b2c_s[:], b2c_d[:])
            nc.sync.dma_start(bl1_s[:], bl1_d[:])
            nc.sync.dma_start(bl2_s[:], bl2_d[:])

            nc.sync.dma_start(dinv128_s[:], dinv128_d[:])

            xT = pool.tile([cfg.in_ch, NPC], FP16)
            nc.sync.dma_start(xT[:], xT_d[:])
            W1 = pool.tile([cfg.in_ch, H], FP16)
            nc.sync.dma_start(W1[:], W1_d[:])
            make_table_tiles(pool, psum, xT, W1, t1s_d, 0, cfg.ntile128)

    nc.gpsimd.collective_compute(
        "AllGather", Alu.bypass, replica_groups=[list(range(N_CORES))],
        ins=[t1s_d[:]], outs=[t1c_d[:]]).then_inc(cc_sem, 1)
    nc.gpsimd.wait_ge(cc_sem, 1)

    # ---------------- Phase 2: aggregate L1 (+T2 build) ----------------
    with tile.TileContext(nc) as tc:
        with tc.tile_pool(name="p2b", bufs=2) as bigpool, \
             tc.tile_pool(name="p2g", bufs=3) as gpool, \
             tc.tile_pool(name="p2", bufs=3) as pool, \
             tc.tile_pool(name="p2h", bufs=1) as hpool, \
             tc.tile_pool(name="p2ps", bufs=2, space="PSUM") as psum:
            h1T = hpool.tile([H, NPC], FP16)
            agg_layer(gpool, bigpool, pool, psum, t1c_d, b1c_s, h1T, layer=1)

    nc.gpsimd.collective_compute(
        "AllGather", Alu.bypass, replica_groups=[list(range(N_CORES))],
        ins=[t2s_d[:]], outs=[t2c_d[:]]).then_inc(cc_sem, 1)
    nc.gpsimd.wait_ge(cc_sem, 2)

    # ---------------- Phase 3: aggregate L2 + MLP ----------------
    with tile.TileContext(nc) as tc:
        with tc.tile_pool(name="p3b", bufs=2) as bigpool, \
             tc.tile_pool(name="p3g", bufs=3) as gpool, \
             tc.tile_pool(name="p3", bufs=3) as pool, \
             tc.tile_pool(name="p3h", bufs=1) as hpool, \
             tc.tile_pool(name="p3ps", bufs=2, space="PSUM") as psum:
            h2T = hpool.tile([H, NPC], FP16)
            agg_layer(gpool, bigpool, pool, psum, t2c_d, b2c_s, h2T, layer=2)

    nc.compile()
    return nc


_CACHE = {}


def _get_program(key, cfg):
    if key not in _CACHE:
        _CACHE[key] = build_program(cfg)
    return _CACHE[key]


def kernel(x, edge_index, W1, b1, W2, b2, Wl1, bl1, Wl2, bl2):
    x = np.asarray(x)
    n = x.shape[0]
    cfg, in_maps, node_of = host_prep(x, edge_index, W1, b1, W2, b2,
                                      Wl1, bl1, Wl2, bl2)
    key = (n, cfg.in_ch, cfg.hid, cfg.bcols)
    nc = _get_program(key, cfg)
    res = run_bass_kernel_spmd(nc, in_maps, list(range(N_CORES)))
    ys = [res.results[c]["y"].reshape(-1) for c in range(N_CORES)]
    y_slots = np.concatenate(ys)
    # slot s holds output of node node_of[s]
    y = np.empty(n, dtype=np.float32)
    valid = node_of < n
    y[node_of[valid]] = y_slots[valid]
    return y.reshape(n, 1)
